# revision 13
# baseline (speedup 1.0000x reference)
"""Causal self-attention Trainium2 kernel (B=4, T=2048, C=1024, H=16, D=64).

Sharding: 8 cores = 4 batches x 2 head-groups (8 heads each).  Partial
c_proj outputs of the two head-group cores of a batch are summed on the
host; b_proj and the (b_attn v-slice @ w_proj) fold are added there too.

Numerics / speed strategy (cost-model driven):
  * QKV is computed with fp8(e4m3) DoubleRow matmuls contracting 2x128
    packed c-chunks per instruction.  Full accuracy is recovered with a
    hi/lo split: qkv ~= x8@w8 + xlo8@w8 + x8@wlo8 (the lo terms carry the
    fp8 rounding residue of x and 64*w; the dropped lo@lo term is ~1e-3
    relative).  All three passes accumulate in one f32 PSUM group.
  * Weights are pre-scaled by 64 on the host so fp8 sees ~N(0,1) values;
    the 64x on q,k is folded into the softmax exp scale (1/(8*4096)) and
    the 64x on v cancels against a 64-valued denominator column.
  * Scores S^T use fp8 DoubleRow too: q^T,k^T are written (via a
    host-side weight-column permutation) straight into a [32, 2-slot, T]
    layout where slot = d/32, so one DR matmul contracts the full D=64.
  * P@V runs in bf16 with V as the moving operand: out[q,65] per
    (head, q-subtile) accumulated over k-tiles; column 64 (denominator)
    comes from a 64.0-filled column appended to V.
  * exp runs on ACT; attention for t-stage i is interleaved with the
    QKV matmuls of stage i+1 so the ~146us of ACT exp overlaps the
    PE-heavy QKV work instead of serializing after it.
  * c_proj is a bf16 matmul from PE-transposed y^T; outputs DMA
    straight from PSUM.
"""

import numpy as np
import ml_dtypes

import concourse.bass as bass
import concourse.tile as tile
import concourse.mybir as mybir
from concourse import bacc, bass_utils

F32 = mybir.dt.float32
BF16 = mybir.dt.bfloat16
F8 = mybir.dt.float8e4
AF = mybir.ActivationFunctionType
DR = mybir.MatmulPerfMode.DoubleRow

NP_F8 = ml_dtypes.float8_e4m3
NP_BF16 = ml_dtypes.bfloat16

B, T, C = 4, 2048, 1024
H = 16            # total heads
HG = 8            # heads per core
D = 64
WS = 64.0         # host-side weight scale for fp8
EXP_SCALE = 1.0 / (8.0 * WS * WS)   # 1/math.sqrt(D) and both 64x factors
NCI = C // 128    # 8 contraction tiles
NTT = T // 128    # 16 t tiles
NTB = 4           # t stages of 512

_NC_CACHE = {}


def build_kernel():
    nc = bacc.Bacc("TRN2", target_bir_lowering=False, debug=False)
    x8_d = nc.dram_tensor("x8", [T, C], F8, kind="ExternalInput").ap()
    xl8_d = nc.dram_tensor("xl8", [T, C], F8, kind="ExternalInput").ap()
    w8_d = nc.dram_tensor("w8", [C, 1536], F8, kind="ExternalInput").ap()
    wl8_d = nc.dram_tensor("wl8", [C, 1536], F8, kind="ExternalInput").ap()
    bqk_d = nc.dram_tensor("bqk", [128, 8], F32, kind="ExternalInput").ap()
    wo_d = nc.dram_tensor("wo", [512, C], BF16, kind="ExternalInput").ap()
    mask_d = nc.dram_tensor("mask", [128, 128], BF16, kind="ExternalInput").ap()
    id8_d = nc.dram_tensor("ident8", [128, 128], F8, kind="ExternalInput").ap()
    idb_d = nc.dram_tensor("identb", [128, 128], BF16, kind="ExternalInput").ap()
    out_d = nc.dram_tensor("out", [T, C], F32, kind="ExternalOutput").ap()

    with tile.TileContext(nc) as tc:
        with tc.tile_pool(name="persist", bufs=1) as persist:
            ident8 = persist.tile([128, 128], F8)
            identb = persist.tile([128, 128], BF16)
            mask_sb = persist.tile([128, 128], BF16)
            bqk_sb = persist.tile([128, 8], F32)
            nc.sync.dma_start(ident8[:], id8_d)
            nc.sync.dma_start(identb[:], idb_d)
            nc.sync.dma_start(mask_sb[:], mask_d)
            nc.sync.dma_start(bqk_sb[:], bqk_d)

            # transposed fp8 activations [128, ci, t]
            xT8 = persist.tile([128, NCI, T], F8)
            xlT8 = persist.tile([128, NCI, T], F8)
            # q^T/k^T fp8 in DoubleRow layout: per quad of 4 heads,
            # partition = (head%4)*32 + d%32, slot = d//32.
            q8s = [persist.tile([128, 2, T], F8, name=f"q8s{q}")
                   for q in range(2)]
            k8s = [persist.tile([128, 2, T], F8, name=f"k8s{q}")
                   for q in range(2)]
            # [64*V | 64] per k-tile: [tt][h*65:(h+1)*65]
            v_all = persist.tile([128, NTT, 520], BF16)
            # y [q-part, tt, head*64+d] and y^T [c-part, cc, t]
            y_sb = persist.tile([128, NTT, 512], BF16)
            yT = persist.tile([128, 4, T], BF16)
            # weights
            w8sb = persist.tile([128, NCI, 1536], F8)
            wl8sb = persist.tile([128, NCI, 1536], F8)
            wo_sb = persist.tile([128, 4, C], BF16)
            nc.gpsimd.dma_start(
                w8sb[:], w8_d.rearrange("(ci p) co -> p ci co", p=128))
            nc.gpsimd.dma_start(
                wl8sb[:], wl8_d.rearrange("(ci p) co -> p ci co", p=128))
            nc.gpsimd.dma_start(
                wo_sb[:], wo_d.rearrange("(cc p) co -> p cc co", p=128))
            nc.gpsimd.memset(v_all[:], WS)

            # ------------- phase 1: transpose x8, xl8 ----------------------
            with (
                tc.tile_pool(name="xnat", bufs=1) as xnat_pool,
                tc.tile_pool(name="pst", bufs=2, space="PSUM") as pst_pool,
            ):
                xnat8 = xnat_pool.tile([128, NTT, C], F8, tag="x")
                xlnat8 = xnat_pool.tile([128, NTT, C], F8, tag="xl")
                nc.sync.dma_start(
                    xnat8[:], x8_d.rearrange("(tt p) c -> p tt c", p=128))
                nc.sync.dma_start(
                    xlnat8[:], xl8_d.rearrange("(tt p) c -> p tt c", p=128))
                for src, dst in ((xnat8, xT8), (xlnat8, xlT8)):
                    for ci in range(NCI):
                        # fp8 transpose mode requires output element step 2
                        pst = pst_pool.tile([128, T, 2], F8, tag="pst")
                        for tt in range(NTT):
                            nc.tensor.transpose(
                                pst[:, tt * 128:(tt + 1) * 128, 0],
                                src[:, tt, ci * 128:(ci + 1) * 128],
                                ident8[:],
                            )
                        nc.vector.tensor_copy(dst[:, ci, :], pst[:, :, 0])

            # ------------- phase 2+3+4: pipelined QKV / attention / proj ---
            with (
                tc.tile_pool(name="psq", bufs=2, space="PSUM") as psq_pool,
                tc.tile_pool(name="pss", bufs=2, space="PSUM") as pss_pool,
                tc.tile_pool(name="pso", bufs=1, space="PSUM") as pso_pool,
                tc.tile_pool(name="ee", bufs=6) as ee_pool,
                tc.tile_pool(name="norm", bufs=8) as norm_pool,
                tc.tile_pool(name="io", bufs=4) as io_pool,
            ):
                def qkv_accum(ps, rhs_of, n):
                    """12 DoubleRow matmuls: x8@w8 + xl8@w8 + x8@wl8."""
                    idx = 0
                    for xs, ws in ((xT8, w8sb), (xlT8, w8sb), (xT8, wl8sb)):
                        for cp in range(4):
                            nc.tensor.matmul(
                                ps[:],
                                *rhs_of(xs, ws, cp, n),
                                start=(idx == 0), stop=(idx == 11),
                                perf_mode=DR,
                            )
                            idx += 1

                def emit_v(tt):
                    ps = psq_pool.tile([128, 512], F32, tag="psq")
                    qkv_accum(
                        ps,
                        lambda xs, ws, cp, n: (
                            xs[:, 2 * cp:2 * cp + 2, n * 128:(n + 1) * 128],
                            ws[:, 2 * cp:2 * cp + 2, 1024:1536],
                        ),
                        tt,
                    )
                    vrow = v_all[:, tt, :].rearrange("p (h x) -> p h x", x=65)
                    nc.vector.tensor_copy(
                        vrow[:, :, 0:64],
                        ps[:].rearrange("p (h d) -> p h d", d=64),
                    )

                def emit_qk(ct, tb):
                    dst = (q8s if ct < 4 else k8s)[(ct % 4) // 2]
                    slot = ct % 2
                    ps = psq_pool.tile([128, 512], F32, tag="psq")
                    qkv_accum(
                        ps,
                        lambda xs, ws, cp, n: (
                            ws[:, 2 * cp:2 * cp + 2, n * 128:(n + 1) * 128],
                            xs[:, 2 * cp:2 * cp + 2,
                               tb * 512:(tb + 1) * 512],
                        ),
                        ct,
                    )
                    nc.vector.tensor_add(
                        dst[:, slot, tb * 512:(tb + 1) * 512],
                        ps[:],
                        bqk_sb[:, ct:ct + 1].to_broadcast([128, 512]),
                    )

                def emit_yt(cc, tbb):
                    pst_f = psq_pool.tile([128, 512], F32, tag="psq")
                    pst = pst_f.bitcast(BF16)
                    for tl in range(4):
                        tt = tbb * 4 + tl
                        nc.tensor.transpose(
                            pst[:, tl * 128:(tl + 1) * 128],
                            y_sb[:, tt, cc * 128:(cc + 1) * 128],
                            identb[:],
                        )
                    nc.vector.tensor_copy(
                        yT[:, cc, tbb * 512:(tbb + 1) * 512], pst[:, 0:512])

                osb_tiles = {}

                def emit_proj(tt, nb):
                    po = psq_pool.tile([128, 512], F32, tag="psq")
                    for cc in range(4):
                        nc.tensor.matmul(
                            po[:],
                            yT[:, cc, tt * 128:(tt + 1) * 128],
                            wo_sb[:, cc, nb * 512:(nb + 1) * 512],
                            start=(cc == 0), stop=(cc == 3),
                        )
                    if tt not in osb_tiles:
                        osb_tiles[tt] = io_pool.tile(
                            [128, 1024], F32, tag="osb", name=f"osb{tt}")
                    osb = osb_tiles[tt]
                    nc.scalar.copy(osb[:, nb * 512:(nb + 1) * 512], po[:])
                    if nb == 1:
                        nc.sync.dma_start(
                            out_d[tt * 128:(tt + 1) * 128, :], osb[:])
                        del osb_tiles[tt]

                # filler units: (cost_ns, emit_fn); queued per stage
                import collections
                filler_q = collections.deque()
                clock = [0.0, 0.0]   # pe, act virtual time

                def pull_fillers(margin=600.0):
                    while filler_q and clock[0] + margin < clock[1]:
                        cost, fn = filler_q.popleft()
                        fn()
                        clock[0] += cost

                def drain_fillers(upto_kind=None):
                    while filler_q:
                        cost, fn = filler_q.popleft()
                        fn()
                        clock[0] += cost

                def attention(u, i):
                    njt = 4 * i + 4
                    hA, hB = 2 * u, 2 * u + 1
                    qd = hA // 4
                    pA, pB = 32 * (hA % 4), 32 * (hB % 4)
                    oa = pso_pool.tile([128, 260], F32, tag="oa",
                                       padded_shape=[128, 512])
                    ob = pso_pool.tile([128, 260], F32, tag="ob",
                                       padded_shape=[128, 512])
                    ees = []
                    for j in range(njt):
                        m = j - 4 * i
                        off = max(m, 0) * 128
                        ps = pss_pool.tile([128, 1024], F32, tag="ps")
                        nc.tensor.matmul(
                            ps[:, off:512],
                            k8s[qd][pA:pA + 32, :, j * 128:(j + 1) * 128],
                            q8s[qd][pA:pA + 32, :,
                                    i * 512 + off:(i + 1) * 512],
                            start=True, stop=True, perf_mode=DR,
                            tile_position=(pA, 0),
                        )
                        nc.tensor.matmul(
                            ps[:, 512:1024 - off],
                            k8s[qd][pB:pB + 32, :, j * 128:(j + 1) * 128],
                            q8s[qd][pB:pB + 32, :,
                                    i * 512 + off:(i + 1) * 512],
                            start=True, stop=True, perf_mode=DR,
                            tile_position=(pB, 0),
                        )
                        clock[0] += (512 - off) * 0.42
                        ee = ee_pool.tile([128, 1024], BF16, tag="E")
                        nc.scalar.activation(
                            ee[:, off:1024 - off], ps[:, off:1024 - off],
                            AF.Exp, scale=EXP_SCALE)
                        clock[1] += (1024 - 2 * off 
                                     + 250) * 0.833
                        if m >= 0:
                            nc.gpsimd.tensor_mul(
                                ee[:, off:off + 128],
                                ee[:, off:off + 128], mask_sb[:])
                            nc.gpsimd.tensor_mul(
                                ee[:, 512:640],
                                ee[:, 512:640], mask_sb[:])
                        ees.append((ee, off))
                        # PV for this j
                        for qs in range(max(m, 0), 4):
                            for po, hh in ((oa, hA), (ob, hB)):
                                st0 = (qs * 128 if po is oa
                                       else 512 + qs * 128 - off)
                                nc.tensor.matmul(
                                    po[:, qs * 65:qs * 65 + 65],
                                    ee[:, st0:st0 + 128],
                                    v_all[:, j, hh * 65:hh * 65 + 65],
                                    start=(j == 0 and qs == 0),
                                    stop=(j == 4 * i + qs),
                                    skip_group_check=True,
                                )
                        clock[0] += (4 - max(m, 0)) * 2 * 65 * 0.42
                        if m >= 0:
                            qs = m
                            for po, hh in ((oa, hA), (ob, hB)):
                                rc = norm_pool.tile([128, 1], F32, tag="rc")
                                nc.vector.reciprocal(
                                    rc[:],
                                    po[:, qs * 65 + 64:qs * 65 + 65])
                                nc.vector.tensor_mul(
                                    y_sb[:, 4 * i + qs,
                                         hh * 64:hh * 64 + 64],
                                    po[:, qs * 65:qs * 65 + 64],
                                    rc[:].to_broadcast([128, 64]),
                                )
                        pull_fillers()

                # ---- pipelined emission ------------------------------
                # stage 0 qkv upfront
                for tl in range(4):
                    emit_v(tl)
                for ct in range(8):
                    emit_qk(ct, 0)
                clock[0] += 12 * 1280.0

                for i in range(NTB):
                    # queue next stage's qkv as filler
                    if i + 1 < NTB:
                        tb = i + 1
                        for tl in range(4):
                            filler_q.append(
                                (1280.0, (lambda n=tb * 4 + tl:
                                          emit_v(n))))
                        for ct in range(8):
                            filler_q.append(
                                (1280.0, (lambda c=ct, t=tb: emit_qk(c, t))))
                    # queue previous stage's y^T transpose + projection
                    if i >= 1:
                        tbb = i - 1
                        for cc in range(4):
                            filler_q.append(
                                (280.0, (lambda c=cc, t=tbb:
                                         emit_yt(c, t))))
                        for tl in range(4):
                            for nb in range(2):
                                filler_q.append(
                                    (900.0, (lambda t=tbb * 4 + tl, n=nb:
                                             emit_proj(t, n))))
                    for u in range(4):
                        attention(u, i)
                    # next stage's qkv must exist before att(*, i+1)
                    drain_fillers()

                # tail: stages 2,3 projection
                for tbb in (2, 3):
                    for cc in range(4):
                        emit_yt(cc, tbb)
                    for tl in range(4):
                        for nb in range(2):
                            emit_proj(tbb * 4 + tl, nb)

    nc.compile()
    return nc


def _get_nc():
    if "nc" not in _NC_CACHE:
        _NC_CACHE["nc"] = build_kernel()
    return _NC_CACHE["nc"]


def _f8(a):
    return np.asarray(a, np.float32).astype(NP_F8)


def _make_in_maps(inputs):
    """Build the 8 per-core input dicts from the full (unsharded) inputs."""
    x = np.asarray(inputs["x"], dtype=np.float32)
    w_attn = np.asarray(inputs["w_attn"], dtype=np.float32)
    b_attn = np.asarray(inputs["b_attn"], dtype=np.float32)
    w_proj = np.asarray(inputs["w_proj"], dtype=np.float32)

    p = np.arange(128)
    mask = (p[None, :] >= p[:, None]).astype(NP_BF16)
    ident = np.eye(128)
    in_maps = []
    for core in range(8):
        b, g = core // 2, core % 2
        x8 = _f8(x[b])
        xl8 = _f8(x[b] - x8.astype(np.float32))
        # column permutation: ct 0..3 q (quad,slot), 4..7 k, 8..11 v plain
        cols = np.empty(1536, np.int64)
        for ct in range(8):
            base = 0 if ct < 4 else C
            quad = (ct % 4) // 2
            slot = ct % 2
            h = quad * 4 + p // 32
            d = slot * 32 + (p % 32)
            cols[ct * 128:(ct + 1) * 128] = base + g * 512 + h * 64 + d
        cols[1024:1536] = 2 * C + g * 512 + np.arange(512)
        ws = w_attn[:, cols] * WS
        w8 = _f8(ws)
        wl8 = _f8(ws - w8.astype(np.float32))
        bqk = (b_attn[cols[:1024]] * WS).reshape(8, 128).T.copy()
        wo = w_proj[g * 512:(g + 1) * 512, :].astype(NP_BF16)
        in_maps.append({
            "x8": x8, "xl8": xl8, "w8": w8, "wl8": wl8,
            "bqk": np.ascontiguousarray(bqk, np.float32), "wo": wo,
            "mask": mask, "ident8": ident.astype(NP_F8),
            "identb": ident.astype(NP_BF16),
        })
    return in_maps


def kernel(x, w_attn, b_attn, w_proj, b_proj):
    b_attn = np.asarray(b_attn, dtype=np.float32)
    b_proj = np.asarray(b_proj, dtype=np.float32)
    w_proj = np.asarray(w_proj, dtype=np.float32)
    in_maps = _make_in_maps({"x": x, "w_attn": w_attn, "b_attn": b_attn,
                             "w_proj": w_proj})
    nc = _get_nc()
    res = bass_utils.run_bass_kernel_spmd(nc, in_maps, core_ids=list(range(8)))
    # v-bias folds linearly through c_proj; add it host-side with b_proj
    host_bias = b_proj + b_attn[2 * C:3 * C] @ w_proj
    out = np.empty((B, T, C), dtype=np.float32)
    for b in range(B):
        out[b] = (res.results[2 * b]["out"] + res.results[2 * b + 1]["out"]
                  + host_bias)
    return out
